# revision 2
# baseline (speedup 1.0000x reference)
"""Causal attention (B=8, T=2048, D=1024) on 8 trn2 NeuronCores .

Data-parallel over batch: core b computes batch element b.

Per-core algorithm (S^T orientation, mixed bf16 / fp8-DoubleRow):
  S^T[k,q] = KT.T @ QT    computed DIRECTLY (k on partitions) so that
                          P^T = exp((S^T)/sqrt(D) - 2) is already the
                          stationary operand for O = P^T.T @ V — no PE
                          transposes, no PSUM->SBUF P copies.
  Causal mask applied by zeroing P^T on the diagonal chunks with a
  GpSimd affine_select (keep c - p - 128j >= 0) instead of -inf adds.
  Row sums from tiny ones-vector matmuls accumulated in PSUM; the
  division uses the SAME quantized P^T so quantization errors cancel.
  exp bias of -2 keeps e^z under fp8e4m3 max (240); softmax is
  shift-invariant so O is unchanged.

  q-tiles 0..3 (rows < 1024: few keys, fp8 noise doesn't average out)
  run in bf16; q-tiles 4..7 run fully in fp8e4m3 with DoubleRow
  (256-deep contraction, 2x matmul throughput).
  Emulated end-to-end error: 8.4e-3 of output scale (gate 2e-2).

Output returned bf16 from device, cast to f32 on host.
"""

import sys

if "/opt/trn_rl_repo" not in sys.path:
    sys.path.insert(0, "/opt/trn_rl_repo")

import numpy as np

B, T, D = 8, 2048, 1024
QW = 256           # q-tile width
NBF = 3            # q-tiles 0..NBF-1 keep P/V in bf16, rest fp8-DR
NEG = -1e10
SOFTMAX_SCALE = 1.0 / 32.0
EXP_BIAS = -2.0

_CACHE = {}


def _split_waits(nc):
    """Walrus here accepts only ONE sync-wait per instruction; hoist extras
    onto same-engine NoOps (engine streams execute in order)."""
    import concourse.mybir as mybir

    n_split = 0
    for f in nc.m.functions:
        for bb in f.blocks:
            out = []
            for inst in bb.instructions:
                si = inst.sync_info
                if si is not None and len(si.on_wait) > 1:
                    waits = list(si.on_wait)
                    for w in waits[:-1]:
                        nop = mybir.InstNoOp(
                            name=f"{inst.name}-w{n_split}",
                            engine=inst.engine,
                            sync_info=mybir.SyncInfo(on_wait=[w], on_update=[]),
                            bass_nofuse=True,
                        )
                        out.append(nop)
                        n_split += 1
                    inst.sync_info = mybir.SyncInfo(
                        on_wait=[waits[-1]], on_update=list(si.on_update)
                    )
                out.append(inst)
            bb.instructions[:] = out
    return n_split


def _build():
    import concourse.bass as bass
    import concourse.mybir as mybir
    import concourse.tile as tile

    f32 = mybir.dt.float32
    bf16 = mybir.dt.bfloat16
    fp8 = mybir.dt.float8e4
    EXP = mybir.ActivationFunctionType.Exp
    DR = mybir.MatmulPerfMode.DoubleRow
    GE = mybir.AluOpType.is_ge

    nc = bass.Bass()
    # fp8 DR layouts: [p, dsc, j, t] = X[t, dsc*256 + j*128 + p]
    kt8_d = nc.dram_tensor("kt8", [128, 4, 2, T], fp8, kind="ExternalInput")
    # q >= 256 (all tiles but 0 use fp8 scores): t index = q - 256
    qt8_d = nc.dram_tensor("qt8", [128, 4, 2, T - QW], fp8, kind="ExternalInput")
    # [p, ksc, j, d] = V[ksc*256 + j*128 + p, d]
    v8_d = nc.dram_tensor("v8", [128, 8, 2, D], fp8, kind="ExternalInput")
    # bf16 layouts for q,k < 256 (tile 0 scores): [p, dc, x] = X[x, dc*128 + p]
    qtb_d = nc.dram_tensor("qtb", [128, 8, QW], bf16, kind="ExternalInput")
    ktb_d = nc.dram_tensor("ktb", [128, 8, QW], bf16, kind="ExternalInput")
    # [p, kc, d] = V[kc*128 + p, d] for k < 1024
    vb_d = nc.dram_tensor("vb", [128, 6, D], bf16, kind="ExternalInput")
    o_d = nc.dram_tensor("out", [T, D], bf16, kind="ExternalOutput")

    with tile.TileContext(nc) as tc:
        with (
            tc.tile_pool(name="const", bufs=1) as constp,
            tc.tile_pool(name="big", bufs=1) as bigp,
            tc.tile_pool(name="pt", bufs=3) as ptp,
            tc.tile_pool(name="ptb", bufs=4) as ptbp,
            tc.tile_pool(name="ob", bufs=6) as obp,
            tc.tile_pool(name="small", bufs=4) as smallp,
            tc.tile_pool(name="ps_st", bufs=2, space="PSUM") as ps_st,
            tc.tile_pool(name="ps_o", bufs=2, space="PSUM") as ps_o,
            tc.tile_pool(name="ps_rs", bufs=2, space="PSUM") as ps_rs,
        ):
            # ---- constants -------------------------------------------------
            # additive causal masks for the two diagonal k-chunks:
            # cmask[p, j, c] = 0 if c >= p + 128j else NEG
            cmask = constp.tile([128, 2, QW], f32)
            nc.gpsimd.memset(cmask[:], 0.0)
            for j in range(2):
                nc.gpsimd.affine_select(
                    out=cmask[:, j, :], in_=cmask[:, j, :],
                    compare_op=GE, fill=NEG,
                    base=-j * 128, channel_multiplier=-1,
                    pattern=[[1, QW]],
                )
            nbias = constp.tile([128, 1], f32)
            nc.gpsimd.memset(nbias[:], EXP_BIAS)
            ones_f = constp.tile([128, 2, 1], f32)
            nc.gpsimd.memset(ones_f[:], 1.0)
            ones8 = constp.tile([128, 2, 1], fp8)
            nc.vector.tensor_copy(ones8[:], ones_f[:])
            onesb = constp.tile([128, 1], bf16)
            nc.vector.tensor_copy(onesb[:], ones_f[:, 0, :])
            heat_f = constp.tile([128, 128], f32)
            nc.vector.memset(heat_f[:], 1.0)
            heat = constp.tile([128, 128], bf16)
            nc.vector.tensor_copy(heat[:], heat_f[:])

            # ---- resident inputs ------------------------------------------
            kt8 = bigp.tile([128, 4, 2, T], fp8)
            qt8 = bigp.tile([128, 4, 2, T - QW], fp8)
            v8 = bigp.tile([128, 8, 2, D], fp8)
            qtb = bigp.tile([128, 8, QW], bf16)
            ktb = bigp.tile([128, 8, QW], bf16)
            vb = bigp.tile([128, 6, D], bf16)

            # One serial input stream on the Sync HWDGE ring in strict
            # need-order (each DMA instr occupies the ring for its transfer;
            # rings contend for HBM, so global ordering beats parallel rings).
            # Outputs go out on the GpSimd SWDGE ring, which is otherwise idle.
            nc.sync.dma_start(ktb[:], ktb_d[:, :, :])
            nc.sync.dma_start(qtb[:], qtb_d[:, :, :])
            nc.sync.dma_start(vb[:, 0:2, :], vb_d[:, 0:2, :])
            nc.sync.dma_start(kt8[:, :, :, 0:512], kt8_d[:, :, :, 0:512])
            nc.sync.dma_start(qt8[:, :, :, 0:256], qt8_d[:, :, :, 0:256])
            nc.sync.dma_start(vb[:, 2:4, :], vb_d[:, 2:4, :])
            nc.sync.dma_start(kt8[:, :, :, 512:1024], kt8_d[:, :, :, 512:1024])
            nc.sync.dma_start(qt8[:, :, :, 256:768], qt8_d[:, :, :, 256:768])
            nc.sync.dma_start(vb[:, 4:6, :], vb_d[:, 4:6, :])
            nc.sync.dma_start(kt8[:, :, :, 1024:2048], kt8_d[:, :, :, 1024:2048])
            nc.sync.dma_start(qt8[:, :, :, 768:1792], qt8_d[:, :, :, 768:1792])
            nc.sync.dma_start(v8[:], v8_d[:, :, :, :])

            # ---- PE heater: warm the HAM clock gate during first DMAs -----
            heat_ps = ps_o.tile([128, 1024], f32, tag="o")
            for i in range(30):
                nc.tensor.matmul(heat_ps[:, :128], heat[:], heat[:],
                                 start=True, stop=True)

            def _finalize(qt, qh, o_ps, rs_ps):
                rinv = smallp.tile([128, 1], f32, tag="rinv")
                nc.vector.reciprocal(rinv[:], rs_ps[:, :1])
                ob = obp.tile([128, D], bf16, tag="ob")
                nc.vector.tensor_scalar_mul(
                    ob[:, 0:512], o_ps[:, 0:512], rinv[:])
                nc.scalar.mul(
                    ob[:, 512:1024], o_ps[:, 512:1024], rinv[:])
                q0 = qt * QW + qh * 128
                nc.gpsimd.dma_start(o_d[q0:q0 + 128, :], ob[:])

            # ---- q-tiles 0..NBF-1 (bf16) ----------------------------------
            def emit_qtile_bf16(qt):
                nkc = 2 * qt + 2
                o_ps = [ps_o.tile([128, 1024], f32, tag="o", name=f"ob{qt}_{i}")
                        for i in range(2)]
                rs_ps = [ps_rs.tile([128, 1], f32, tag="rs", name=f"rb{qt}_{i}")
                         for i in range(2)]
                q8off = qt * QW - QW  # qt8 column offset (q >= 256)
                pts = {}

                def emit_st(kc):
                    # last (odd-diagonal) chunk only sees q-cols 128..255; its
                    # left half is never read (qh=0 skips it) 
                    c0 = 128 if kc == nkc - 1 else 0
                    st = ps_st.tile([128, QW], f32, tag="st")
                    if qt == 0:
                        for dc in range(8):
                            nc.tensor.matmul(
                                st[:, c0:QW],
                                ktb[:, dc, kc * 128:(kc + 1) * 128],
                                qtb[:, dc, c0:QW],
                                start=(dc == 0), stop=(dc == 7),
                            )
                    else:
                        for dsc in range(4):
                            nc.tensor.matmul(
                                st[:, c0:QW],
                                kt8[:, dsc, :, kc * 128:(kc + 1) * 128],
                                qt8[:, dsc, :, q8off + c0:q8off + QW],
                                start=(dsc == 0), stop=(dsc == 3),
                                perf_mode=DR,
                            )
                    if kc == nkc - 2:
                        nc.vector.tensor_add(st[:, :QW], st[:, :QW],
                                             cmask[:, 0, :])
                    elif kc == nkc - 1:
                        nc.vector.tensor_add(st[:, 128:QW], st[:, 128:QW],
                                             cmask[:, 1, 128:QW])
                    p = ptbp.tile([128, QW], bf16, tag="ptb")
                    nc.scalar.activation(p[:, c0:QW], st[:, c0:QW], EXP,
                                         bias=nbias[:], scale=SOFTMAX_SCALE)
                    pts[kc] = p

                def emit_o(kc):
                    p = pts.pop(kc)
                    for qh in range(2):
                        if qh == 0 and kc == nkc - 1:
                            continue  # block qh=0 doesn't see the last chunk
                        stop = (kc == nkc - 2) if qh == 0 else (kc == nkc - 1)
                        for h in range(2):
                            nc.tensor.matmul(
                                o_ps[qh][:, h * 512:(h + 1) * 512],
                                p[:, qh * 128:(qh + 1) * 128],
                                vb[:, kc, h * 512:(h + 1) * 512],
                                start=(kc == 0), stop=stop,
                            )
                        nc.tensor.matmul(
                            rs_ps[qh][:, :1],
                            p[:, qh * 128:(qh + 1) * 128],
                            onesb[:],
                            start=(kc == 0), stop=stop,
                        )

                for kc in range(nkc):
                    emit_st(kc)
                    if kc >= 1:
                        emit_o(kc - 1)
                emit_o(nkc - 1)
                for qh in range(2):
                    _finalize(qt, qh, o_ps[qh], rs_ps[qh])

            # ---- q-tiles NBF..7 (fp8 DoubleRow) ---------------------------
            def emit_qtile_fp8(qt):
                nkc = 2 * qt + 2
                q8off = qt * QW - QW  # column offset in qt8 (q >= 256)
                o_ps = [ps_o.tile([128, 1024], f32, tag="o", name=f"o8{qt}_{i}")
                        for i in range(2)]
                rs_ps = [ps_rs.tile([128, 1], f32, tag="rs", name=f"r8{qt}_{i}")
                         for i in range(2)]
                pts = {}

                def emit_st(ksc):
                    pt_pair = ptp.tile([128, 2, QW], fp8, tag="pt")
                    for j in range(2):
                        kc = 2 * ksc + j
                        st = ps_st.tile([128, QW], f32, tag="st")
                        for dsc in range(4):
                            nc.tensor.matmul(
                                st[:, :QW],
                                kt8[:, dsc, :, kc * 128:(kc + 1) * 128],
                                qt8[:, dsc, :, q8off:q8off + QW],
                                start=(dsc == 0), stop=(dsc == 3),
                                perf_mode=DR,
                            )
                        if ksc == qt:
                            # diagonal: -inf where k > q, so exp gives 0
                            nc.vector.tensor_add(st[:, :QW], st[:, :QW],
                                                 cmask[:, j, :])
                        nc.scalar.activation(pt_pair[:, j, :],
                                             st[:, :QW], EXP,
                                             bias=nbias[:], scale=SOFTMAX_SCALE)
                    pts[ksc] = pt_pair

                def emit_o(ksc, order):
                    pt_pair = pts.pop(ksc)
                    start = (ksc == order[0])
                    stop = (ksc == order[-1])
                    for qh in range(2):
                        lhs = pt_pair[:, :, qh * 128:(qh + 1) * 128]
                        for h in range(2):
                            nc.tensor.matmul(
                                o_ps[qh][:, h * 512:(h + 1) * 512],
                                lhs,
                                v8[:, ksc, :, h * 512:(h + 1) * 512],
                                start=start, stop=stop,
                                perf_mode=DR,
                            )
                        nc.tensor.matmul(
                            rs_ps[qh][:, :1], lhs, ones8[:],
                            start=start, stop=stop,
                            perf_mode=DR,
                        )

                order = list(range(qt + 1))
                if qt == 7:
                    # last tile: diagonal first so the final O matmuls don't
                    # wait on the diagonal's mask+exp chain at the very end
                    order = [qt] + list(range(qt))
                for i, ksc in enumerate(order):
                    emit_st(ksc)
                    if i >= 1:
                        emit_o(order[i - 1], order)
                emit_o(order[-1], order)
                for qh in range(2):
                    _finalize(qt, qh, o_ps[qh], rs_ps[qh])

            for qt in range(NBF):
                emit_qtile_bf16(qt)
            for qt in range(NBF, 8):
                emit_qtile_fp8(qt)

    _split_waits(nc)
    return nc


def _np_reference(query, key, value, mask):
    """Host fallback for the general (non-all-ones) padding-mask case."""
    out = np.empty_like(query)
    tri = np.triu(np.ones((T, T), dtype=np.float32), 1) * 1e10
    for b in range(B):
        s = query[b] @ key[b].T
        s = s - tri
        s = s - (1.0 - mask[b])[None, :] * 1e10
        s = s * SOFTMAX_SCALE
        s = s - s.max(axis=-1, keepdims=True)
        p = np.exp(s)
        p = p / p.sum(axis=-1, keepdims=True)
        out[b] = p @ value[b]
    return out


def make_in_maps(query, key, value):
    """Per-core input dicts with host-side relayout + dtype casts."""
    import ml_dtypes

    fp8 = ml_dtypes.float8_e4m3
    bf16 = ml_dtypes.bfloat16
    H = T // 2
    maps = []
    for b in range(B):
        q = query[b]          # [T, D]
        k = key[b]
        v = value[b]
        # fp8 DR: [p, dsc, j, t] = X[t, dsc*256 + j*128 + p]
        kt = np.ascontiguousarray(
            k.T.reshape(4, 2, 128, T).transpose(2, 0, 1, 3)).astype(fp8)
        qt = np.ascontiguousarray(
            q[QW:].T.reshape(4, 2, 128, T - QW).transpose(2, 0, 1, 3)).astype(fp8)
        # [p, ksc, j, d] = V[ksc*256 + j*128 + p, d]
        v8 = np.ascontiguousarray(
            v.reshape(8, 2, 128, D).transpose(2, 0, 1, 3)).astype(fp8)
        # bf16 (q,k < 256, tile 0 scores): [p, dc, x] = X[x, dc*128 + p]
        qtb = np.ascontiguousarray(
            q[:QW].T.reshape(8, 128, QW).transpose(1, 0, 2)).astype(bf16)
        ktb = np.ascontiguousarray(
            k[:QW].T.reshape(8, 128, QW).transpose(1, 0, 2)).astype(bf16)
        # [p, kc, d] = V[kc*128 + p, d]  (k < 768, bf16 O path)
        vb = np.ascontiguousarray(
            v[:768].reshape(6, 128, D).transpose(1, 0, 2)).astype(bf16)
        maps.append({
            "kt8": kt, "qt8": qt, "v8": v8,
            "qtb": qtb, "ktb": ktb, "vb": vb,
        })
    return maps


def kernel(query, key, value, mask):
    query = np.asarray(query, dtype=np.float32)
    key = np.asarray(key, dtype=np.float32)
    value = np.asarray(value, dtype=np.float32)
    mask = np.asarray(mask, dtype=np.float32)

    if not np.all(mask == 1.0):
        return _np_reference(query, key, value, mask)

    from concourse.bass_utils import run_bass_kernel_spmd

    if "nc" not in _CACHE:
        _CACHE["nc"] = _build()
    nc = _CACHE["nc"]

    in_maps = make_in_maps(query, key, value)
    last_err = None
    for _ in range(3):  # retry transient device errors
        try:
            res = run_bass_kernel_spmd(nc, in_maps, core_ids=list(range(B)))
            break
        except Exception as e:  # noqa: BLE001
            last_err = e
    else:
        raise last_err
    out = np.stack(
        [res.results[b]["out"].astype(np.float32) for b in range(B)], axis=0)
    return out


# revision 3
# speedup vs baseline: 1.0872x; 1.0872x over previous
"""Causal attention (B=8, T=2048, D=1024) on 8 trn2 NeuronCores .

Data-parallel over batch: core b computes batch element b.

Per-core algorithm (S^T orientation, mixed bf16 / fp8-DoubleRow):
  S^T[k,q] = KT.T @ QT    computed DIRECTLY (k on partitions) so that
                          P^T = exp((S^T)/sqrt(D) - 2) is already the
                          stationary operand for O = P^T.T @ V — no PE
                          transposes, no PSUM->SBUF P copies.
  Causal mask applied by zeroing P^T on the diagonal chunks with a
  GpSimd affine_select (keep c - p - 128j >= 0) instead of -inf adds.
  Row sums from tiny ones-vector matmuls accumulated in PSUM; the
  division uses the SAME quantized P^T so quantization errors cancel.
  exp bias of -2 keeps e^z under fp8e4m3 max (240); softmax is
  shift-invariant so O is unchanged.

  q-tiles 0..3 (rows < 1024: few keys, fp8 noise doesn't average out)
  run in bf16; q-tiles 4..7 run fully in fp8e4m3 with DoubleRow
  (256-deep contraction, 2x matmul throughput).
  Emulated end-to-end error: 8.4e-3 of output scale (gate 2e-2).

Output returned bf16 from device, cast to f32 on host.
"""

import sys

if "/opt/trn_rl_repo" not in sys.path:
    sys.path.insert(0, "/opt/trn_rl_repo")

import numpy as np

B, T, D = 8, 2048, 1024
QW = 256           # q-tile width
NBF = 3            # q-tiles 0..NBF-1 keep P/V in bf16, rest fp8-DR
NEG = -1e10
SOFTMAX_SCALE = 1.0 / 32.0
EXP_BIAS = -2.0

_CACHE = {}


def _split_waits(nc):
    """Walrus here accepts only ONE sync-wait per instruction; hoist extras
    onto same-engine NoOps (engine streams execute in order)."""
    import concourse.mybir as mybir

    n_split = 0
    for f in nc.m.functions:
        for bb in f.blocks:
            out = []
            for inst in bb.instructions:
                si = inst.sync_info
                if si is not None and len(si.on_wait) > 1:
                    waits = list(si.on_wait)
                    for w in waits[:-1]:
                        nop = mybir.InstNoOp(
                            name=f"{inst.name}-w{n_split}",
                            engine=inst.engine,
                            sync_info=mybir.SyncInfo(on_wait=[w], on_update=[]),
                            bass_nofuse=True,
                        )
                        out.append(nop)
                        n_split += 1
                    inst.sync_info = mybir.SyncInfo(
                        on_wait=[waits[-1]], on_update=list(si.on_update)
                    )
                out.append(inst)
            bb.instructions[:] = out
    return n_split


def _prune_const_memsets(nc):
    """Drop bass-init's 4 built-in const-AP memsets (unused by this kernel).
    They run serially on the GpSimd Q7 (~3.5us) before the init barrier
    releases, delaying everything. Walrus confirms they have no readers."""
    n = 0
    for f in nc.m.functions:
        for bb in f.blocks:
            keep = []
            for inst in bb.instructions:
                outs = getattr(inst, "outs", None) or []
                memref = getattr(outs[0], "memref", "") if outs else ""
                if (type(inst).__name__ == "InstMemset"
                        and isinstance(memref, str)
                        and memref.startswith("const-")
                        and inst.sync_info is None):
                    n += 1
                    continue
                keep.append(inst)
            bb.instructions[:] = keep
    return n


def _build():
    import concourse.bass as bass
    import concourse.mybir as mybir
    import concourse.tile as tile

    f32 = mybir.dt.float32
    bf16 = mybir.dt.bfloat16
    fp8 = mybir.dt.float8e4
    EXP = mybir.ActivationFunctionType.Exp
    DR = mybir.MatmulPerfMode.DoubleRow
    GE = mybir.AluOpType.is_ge

    nc = bass.Bass()
    # fp8 DR layouts: [p, dsc, j, t] = X[t, dsc*256 + j*128 + p]
    kt8_d = nc.dram_tensor("kt8", [128, 4, 2, T], fp8, kind="ExternalInput")
    # q >= 256 (all tiles but 0 use fp8 scores): t index = q - 256
    qt8_d = nc.dram_tensor("qt8", [128, 4, 2, T - QW], fp8, kind="ExternalInput")
    # [p, ksc, j, d] = V[ksc*256 + j*128 + p, d]
    v8_d = nc.dram_tensor("v8", [128, 8, 2, D], fp8, kind="ExternalInput")
    # bf16 layouts for q,k < 256 (tile 0 scores): [p, dc, x] = X[x, dc*128 + p]
    qtb_d = nc.dram_tensor("qtb", [128, 8, QW], bf16, kind="ExternalInput")
    ktb_d = nc.dram_tensor("ktb", [128, 8, QW], bf16, kind="ExternalInput")
    # [p, kc, d] = V[kc*128 + p, d] for k < 1024
    vb_d = nc.dram_tensor("vb", [128, 6, D], bf16, kind="ExternalInput")
    o_d = nc.dram_tensor("out", [T, D], bf16, kind="ExternalOutput")

    with tile.TileContext(nc) as tc:
        with (
            tc.tile_pool(name="const", bufs=1) as constp,
            tc.tile_pool(name="big", bufs=1) as bigp,
            tc.tile_pool(name="pt", bufs=3) as ptp,
            tc.tile_pool(name="ptb", bufs=4) as ptbp,
            tc.tile_pool(name="ob", bufs=6) as obp,
            tc.tile_pool(name="small", bufs=4) as smallp,
            tc.tile_pool(name="ps_st", bufs=2, space="PSUM") as ps_st,
            tc.tile_pool(name="ps_o", bufs=2, space="PSUM") as ps_o,
            tc.tile_pool(name="ps_rs", bufs=2, space="PSUM") as ps_rs,
        ):
            # ---- constants -------------------------------------------------
            # additive causal masks for the two diagonal k-chunks:
            # cmask[p, j, c] = 0 if c >= p + 128j else NEG
            cmask = constp.tile([128, 2, QW], f32)
            nc.gpsimd.memset(cmask[:], 0.0)
            for j in range(2):
                nc.gpsimd.affine_select(
                    out=cmask[:, j, :], in_=cmask[:, j, :],
                    compare_op=GE, fill=NEG,
                    base=-j * 128, channel_multiplier=-1,
                    pattern=[[1, QW]],
                )
            nbias = constp.tile([128, 1], f32)
            nc.gpsimd.memset(nbias[:], EXP_BIAS)
            ones_f = constp.tile([128, 2, 1], f32)
            nc.gpsimd.memset(ones_f[:], 1.0)
            ones8 = constp.tile([128, 2, 1], fp8)
            nc.vector.tensor_copy(ones8[:], ones_f[:])
            onesb = constp.tile([128, 1], bf16)
            nc.vector.tensor_copy(onesb[:], ones_f[:, 0, :])
            heat_f = constp.tile([128, 128], f32)
            nc.vector.memset(heat_f[:], 1.0)
            heat = constp.tile([128, 128], bf16)
            nc.vector.tensor_copy(heat[:], heat_f[:])

            # ---- resident inputs ------------------------------------------
            kt8 = bigp.tile([128, 4, 2, T], fp8)
            qt8 = bigp.tile([128, 4, 2, T - QW], fp8)
            v8 = bigp.tile([128, 8, 2, D], fp8)
            qtb = bigp.tile([128, 8, QW], bf16)
            ktb = bigp.tile([128, 8, QW], bf16)
            vb = bigp.tile([128, 6, D], bf16)

            # One serial input stream on the Sync HWDGE ring in strict
            # need-order (each DMA instr occupies the ring for its transfer;
            # rings contend for HBM, so global ordering beats parallel rings).
            # Outputs go out on the GpSimd SWDGE ring, which is otherwise idle.
            nc.sync.dma_start(ktb[:], ktb_d[:, :, :])
            nc.sync.dma_start(qtb[:], qtb_d[:, :, :])
            nc.sync.dma_start(vb[:, 0:2, :], vb_d[:, 0:2, :])
            nc.sync.dma_start(kt8[:, :, :, 0:512], kt8_d[:, :, :, 0:512])
            nc.sync.dma_start(qt8[:, :, :, 0:256], qt8_d[:, :, :, 0:256])
            nc.sync.dma_start(vb[:, 2:4, :], vb_d[:, 2:4, :])
            nc.sync.dma_start(kt8[:, :, :, 512:1024], kt8_d[:, :, :, 512:1024])
            nc.sync.dma_start(qt8[:, :, :, 256:768], qt8_d[:, :, :, 256:768])
            nc.sync.dma_start(vb[:, 4:6, :], vb_d[:, 4:6, :])
            nc.sync.dma_start(kt8[:, :, :, 1024:2048], kt8_d[:, :, :, 1024:2048])
            nc.sync.dma_start(qt8[:, :, :, 768:1792], qt8_d[:, :, :, 768:1792])
            nc.sync.dma_start(v8[:], v8_d[:, :, :, :])

            # ---- PE heater: warm the HAM clock gate during first DMAs -----
            heat_ps = ps_o.tile([128, 1024], f32, tag="o")
            for i in range(30):
                nc.tensor.matmul(heat_ps[:, :128], heat[:], heat[:],
                                 start=True, stop=True)

            def _finalize(qt, qh, o_ps, rs_ps):
                rinv = smallp.tile([128, 1], f32, tag="rinv")
                nc.vector.reciprocal(rinv[:], rs_ps[:, :1])
                ob = obp.tile([128, D], bf16, tag="ob")
                nc.vector.tensor_scalar_mul(
                    ob[:, 0:512], o_ps[:, 0:512], rinv[:])
                nc.scalar.mul(
                    ob[:, 512:1024], o_ps[:, 512:1024], rinv[:])
                q0 = qt * QW + qh * 128
                nc.gpsimd.dma_start(o_d[q0:q0 + 128, :], ob[:])

            # ---- q-tiles 0..NBF-1 (bf16) ----------------------------------
            def emit_qtile_bf16(qt):
                nkc = 2 * qt + 2
                o_ps = [ps_o.tile([128, 1024], f32, tag="o", name=f"ob{qt}_{i}")
                        for i in range(2)]
                rs_ps = [ps_rs.tile([128, 1], f32, tag="rs", name=f"rb{qt}_{i}")
                         for i in range(2)]
                q8off = qt * QW - QW  # qt8 column offset (q >= 256)
                pts = {}

                def emit_st(kc):
                    # last (odd-diagonal) chunk only sees q-cols 128..255; its
                    # left half is never read (qh=0 skips it) 
                    c0 = 128 if kc == nkc - 1 else 0
                    st = ps_st.tile([128, QW], f32, tag="st")
                    if qt == 0:
                        for dc in range(8):
                            nc.tensor.matmul(
                                st[:, c0:QW],
                                ktb[:, dc, kc * 128:(kc + 1) * 128],
                                qtb[:, dc, c0:QW],
                                start=(dc == 0), stop=(dc == 7),
                            )
                    else:
                        for dsc in range(4):
                            nc.tensor.matmul(
                                st[:, c0:QW],
                                kt8[:, dsc, :, kc * 128:(kc + 1) * 128],
                                qt8[:, dsc, :, q8off + c0:q8off + QW],
                                start=(dsc == 0), stop=(dsc == 3),
                                perf_mode=DR,
                            )
                    if kc == nkc - 2:
                        nc.vector.tensor_add(st[:, :QW], st[:, :QW],
                                             cmask[:, 0, :])
                    elif kc == nkc - 1:
                        nc.vector.tensor_add(st[:, 128:QW], st[:, 128:QW],
                                             cmask[:, 1, 128:QW])
                    p = ptbp.tile([128, QW], bf16, tag="ptb")
                    nc.scalar.activation(p[:, c0:QW], st[:, c0:QW], EXP,
                                         bias=nbias[:], scale=SOFTMAX_SCALE)
                    pts[kc] = p

                def emit_o(kc):
                    p = pts.pop(kc)
                    for qh in range(2):
                        if qh == 0 and kc == nkc - 1:
                            continue  # block qh=0 doesn't see the last chunk
                        stop = (kc == nkc - 2) if qh == 0 else (kc == nkc - 1)
                        for h in range(2):
                            nc.tensor.matmul(
                                o_ps[qh][:, h * 512:(h + 1) * 512],
                                p[:, qh * 128:(qh + 1) * 128],
                                vb[:, kc, h * 512:(h + 1) * 512],
                                start=(kc == 0), stop=stop,
                            )
                        nc.tensor.matmul(
                            rs_ps[qh][:, :1],
                            p[:, qh * 128:(qh + 1) * 128],
                            onesb[:],
                            start=(kc == 0), stop=stop,
                        )

                for kc in range(nkc):
                    emit_st(kc)
                    if kc >= 1:
                        emit_o(kc - 1)
                emit_o(nkc - 1)
                for qh in range(2):
                    _finalize(qt, qh, o_ps[qh], rs_ps[qh])

            # ---- q-tiles NBF..7 (fp8 DoubleRow) ---------------------------
            def emit_qtile_fp8(qt):
                nkc = 2 * qt + 2
                q8off = qt * QW - QW  # column offset in qt8 (q >= 256)
                o_ps = [ps_o.tile([128, 1024], f32, tag="o", name=f"o8{qt}_{i}")
                        for i in range(2)]
                rs_ps = [ps_rs.tile([128, 1], f32, tag="rs", name=f"r8{qt}_{i}")
                         for i in range(2)]
                pts = {}

                def emit_st(ksc):
                    pt_pair = ptp.tile([128, 2, QW], fp8, tag="pt")
                    for j in range(2):
                        kc = 2 * ksc + j
                        st = ps_st.tile([128, QW], f32, tag="st")
                        for dsc in range(4):
                            nc.tensor.matmul(
                                st[:, :QW],
                                kt8[:, dsc, :, kc * 128:(kc + 1) * 128],
                                qt8[:, dsc, :, q8off:q8off + QW],
                                start=(dsc == 0), stop=(dsc == 3),
                                perf_mode=DR,
                            )
                        if ksc == qt:
                            # diagonal: -inf where k > q, so exp gives 0
                            nc.vector.tensor_add(st[:, :QW], st[:, :QW],
                                                 cmask[:, j, :])
                        nc.scalar.activation(pt_pair[:, j, :],
                                             st[:, :QW], EXP,
                                             bias=nbias[:], scale=SOFTMAX_SCALE)
                    pts[ksc] = pt_pair

                def emit_o(ksc, order):
                    pt_pair = pts.pop(ksc)
                    start = (ksc == order[0])
                    stop = (ksc == order[-1])
                    for qh in range(2):
                        lhs = pt_pair[:, :, qh * 128:(qh + 1) * 128]
                        for h in range(2):
                            nc.tensor.matmul(
                                o_ps[qh][:, h * 512:(h + 1) * 512],
                                lhs,
                                v8[:, ksc, :, h * 512:(h + 1) * 512],
                                start=start, stop=stop,
                                perf_mode=DR,
                            )
                        nc.tensor.matmul(
                            rs_ps[qh][:, :1], lhs, ones8[:],
                            start=start, stop=stop,
                            perf_mode=DR,
                        )

                order = list(range(qt + 1))
                if qt == 7:
                    # last tile: diagonal first so the final O matmuls don't
                    # wait on the diagonal's mask+exp chain at the very end
                    order = [qt] + list(range(qt))
                for i, ksc in enumerate(order):
                    emit_st(ksc)
                    if i >= 1:
                        emit_o(order[i - 1], order)
                emit_o(order[-1], order)
                for qh in range(2):
                    _finalize(qt, qh, o_ps[qh], rs_ps[qh])

            for qt in range(NBF):
                emit_qtile_bf16(qt)
            for qt in range(NBF, 8):
                emit_qtile_fp8(qt)

    _prune_const_memsets(nc)
    _split_waits(nc)
    return nc


def _np_reference(query, key, value, mask):
    """Host fallback for the general (non-all-ones) padding-mask case."""
    out = np.empty_like(query)
    tri = np.triu(np.ones((T, T), dtype=np.float32), 1) * 1e10
    for b in range(B):
        s = query[b] @ key[b].T
        s = s - tri
        s = s - (1.0 - mask[b])[None, :] * 1e10
        s = s * SOFTMAX_SCALE
        s = s - s.max(axis=-1, keepdims=True)
        p = np.exp(s)
        p = p / p.sum(axis=-1, keepdims=True)
        out[b] = p @ value[b]
    return out


def make_in_maps(query, key, value):
    """Per-core input dicts with host-side relayout + dtype casts."""
    import ml_dtypes

    fp8 = ml_dtypes.float8_e4m3
    bf16 = ml_dtypes.bfloat16
    H = T // 2
    maps = []
    for b in range(B):
        q = query[b]          # [T, D]
        k = key[b]
        v = value[b]
        # fp8 DR: [p, dsc, j, t] = X[t, dsc*256 + j*128 + p]
        kt = np.ascontiguousarray(
            k.T.reshape(4, 2, 128, T).transpose(2, 0, 1, 3)).astype(fp8)
        qt = np.ascontiguousarray(
            q[QW:].T.reshape(4, 2, 128, T - QW).transpose(2, 0, 1, 3)).astype(fp8)
        # [p, ksc, j, d] = V[ksc*256 + j*128 + p, d]
        v8 = np.ascontiguousarray(
            v.reshape(8, 2, 128, D).transpose(2, 0, 1, 3)).astype(fp8)
        # bf16 (q,k < 256, tile 0 scores): [p, dc, x] = X[x, dc*128 + p]
        qtb = np.ascontiguousarray(
            q[:QW].T.reshape(8, 128, QW).transpose(1, 0, 2)).astype(bf16)
        ktb = np.ascontiguousarray(
            k[:QW].T.reshape(8, 128, QW).transpose(1, 0, 2)).astype(bf16)
        # [p, kc, d] = V[kc*128 + p, d]  (k < 768, bf16 O path)
        vb = np.ascontiguousarray(
            v[:768].reshape(6, 128, D).transpose(1, 0, 2)).astype(bf16)
        maps.append({
            "kt8": kt, "qt8": qt, "v8": v8,
            "qtb": qtb, "ktb": ktb, "vb": vb,
        })
    return maps


def kernel(query, key, value, mask):
    query = np.asarray(query, dtype=np.float32)
    key = np.asarray(key, dtype=np.float32)
    value = np.asarray(value, dtype=np.float32)
    mask = np.asarray(mask, dtype=np.float32)

    if not np.all(mask == 1.0):
        return _np_reference(query, key, value, mask)

    from concourse.bass_utils import run_bass_kernel_spmd

    if "nc" not in _CACHE:
        _CACHE["nc"] = _build()
    nc = _CACHE["nc"]

    in_maps = make_in_maps(query, key, value)
    last_err = None
    for _ in range(3):  # retry transient device errors
        try:
            res = run_bass_kernel_spmd(nc, in_maps, core_ids=list(range(B)))
            break
        except Exception as e:  # noqa: BLE001
            last_err = e
    else:
        raise last_err
    out = np.stack(
        [res.results[b]["out"].astype(np.float32) for b in range(B)], axis=0)
    return out
